# revision 12
# baseline (speedup 1.0000x reference)
"""Distributed causal-attention-with-bias Bass kernel for 8 TRN2 NeuronCores.

Problem (hardcoded): B=4, H=16, S=2048, D=64
  out = softmax(Q K^T / sqrt(D) + bias, causal) @ V

Sharding: core c handles batch b = c//2, heads h in [8*(c%2), 8*(c%2)+8).
Per-(b,h) attention is fully independent; bias[b] is shared by the 8 heads
on a core.

Per core, per head h, window of a k-chunk pair (c0, c0+1):
  psum[k,q]  = K_c @ (ALPHA*Q)^T       (TensorE bf16, paired row groups;
                                        psum lands in Schraudolph units)
  psum[k,q] += 184.66*bias^T[k,q]      (TensorE fp8 DoubleRow: 16*I
                                        stationary x 11.54*bias^T moving;
                                        causal mask baked into the fp8
                                        bias at diagonal blocks.  ACT
                                        windows only)
  ACT windows:  P^T = exp(psum*ln2/128)           (ScalarE, one op)
  DVE windows:  P^T = bitcast(int16(psum + C))    (VectorE, one op;
                                        C = 184.66*bias^T + const from
                                        host; Schraudolph bf16 exp)
  out[q,d+1] += P^T_slice^T @ [V_c|1]  (TensorE; ones col -> denominator)
  out[q,:]    = acc[q,0:64]/acc[q,64]  (VectorE recip+mul from PSUM)

Scheduling: q-blocks processed in 2 passes of 8 (accumulators fit 2 PSUM
banks/pass), freeing 6 banks for 3 score windows in flight; the QK+bias
matmuls of window i+2 are issued to the PE queue BEFORE the PV matmuls of
window i, so the ~2us cross-engine exp chain hides behind PE streaming
and the PE never idles (idle gaps let the HAM clock-gate halve the PE
clock).
"""

import sys

if "/opt/trn_rl_repo" not in sys.path:
    sys.path.insert(0, "/opt/trn_rl_repo")

import math

import ml_dtypes
import numpy as np

import concourse.bass as bass
import concourse.tile as tile
from concourse import bacc, mybir
from concourse.bass_utils import run_bass_kernel_spmd

DT = mybir.dt
AF = mybir.ActivationFunctionType
FP8 = ml_dtypes.float8_e4m3

B, H, S, D = 4, 16, 2048, 64
P = 128              # partition dim / k-chunk size
NCH = S // P         # 16 k-chunks
HPC = H // 2         # 8 heads per core
NCORES = 8
DV = D + 1           # V padded with a ones column

TRACE = False
LAST_EXEC_NS = None
LAST_PROFILE_DIR = None

# Schraudolph constants: with Q prescaled by ALPHA on the host, the QK psum
# is directly in bf16-bit units; adding C = CB_MUL*bias^T + CB_ADD and
# converting to int16 gives the bf16 bit pattern of exp(QK/8 + bias).
ALPHA = 16.0 / math.log(2.0)
ACT_SCALE = math.log(2.0) / 128.0          # psum*ACT_SCALE = QK/8 + bias
CB_MUL = 128.0 / math.log(2.0)             # = 184.66 = 16 * 11.5415
CB_ADD = 128.0 * (127.0 - 0.0579)
BIAS_STAT = 16.0                           # fp8 identity stationary value
BIAS_MOV = CB_MUL / BIAS_STAT              # fp8 bias moving scale
MASK8 = -240.0                             # e4m3 max-negative for causal

# Windows (keyed by (pair_c0, window_start_q)) routed to the fused
# Schraudolph path on VectorE instead of ScalarE exp.  Only windows with
# q >= 1024 (large effective key count -> approximation error averages
# out) and no diagonal blocks are eligible; all have u0 = u1 = 512.
DVE_WINS = ((0, 1024), (0, 1536), (2, 1024), (2, 1536), (4, 1024),
            (4, 1536), (6, 1536), (8, 1536))
DVE_SET = set(DVE_WINS)
CW_OFF = {k: 1024 * i for i, k in enumerate(DVE_WINS)}
CW_TOT = 1024 * len(DVE_WINS)

_built = None


def _nrt_profile_run(nc, in_maps):
    """Run via SPMD with the axon NRT profiler capturing NTFFs, then parse
    core 0's NTFF with neuron-profile to get the NEFF exec time in ns."""
    import ctypes
    import tempfile

    lib = ctypes.CDLL("/opt/axon/libaxon_pjrt.so")
    for f in (lib.axon_start_nrt_profile, lib.axon_stop_nrt_profile):
        f.restype = ctypes.c_int64
        f.argtypes = [ctypes.c_char_p, ctypes.c_size_t]
    d = tempfile.mkdtemp(prefix="attnprof_")
    b = d.encode()
    assert lib.axon_start_nrt_profile(b, len(b)) == 0
    try:
        res = run_bass_kernel_spmd(nc, in_maps, core_ids=list(range(NCORES)))
    finally:
        lib.axon_stop_nrt_profile(b, len(b))
    exec_ns = None
    try:
        from gauge.profiler import FishPath, Profile
        prof = Profile(
            profile_path=FishPath(d), kernel_dev_mode=True,
            profile_on_exit=False, bass_kernel=nc.m,
            offline_processing=True, fname="*_body*",
        )
        prof.convert_ntffs_to_json((0,))
        exec_ns = int(prof.get_total_time(0) * 1e9)
    except Exception as e:  # profiling is best-effort
        print(f"ntff parse failed: {e!r}")
    return res, exec_ns, d


def _pair_windows(c0):
    qs0, qs1 = P * c0, P * (c0 + 1)
    out = []
    for j in range(qs0 // 512, S // 512):
        a0, b0 = max(qs0, 512 * j), 512 * (j + 1)
        a1, b1 = max(qs1, 512 * j), 512 * (j + 1)
        out.append((c0, a0, b0, a1, b1))
    return out


_ALLW = [w for c0 in range(0, NCH, 2) for w in _pair_windows(c0)]


def _interleave(wins):
    """Alternate DVE- and ACT-path windows so the two exp engines take
    turns producing P^T and neither becomes the lone producer (which
    starves the PE and lets the HAM clock-gate cool it)."""
    dve = [w for w in wins if (w[0], w[1]) in DVE_SET]
    act = [w for w in wins if (w[0], w[1]) not in DVE_SET]
    out = []
    while dve or act:
        if dve:
            out.append(dve.pop(0))
        if act:
            out.append(act.pop(0))
    return out


PASS_WINS = [
    [w for w in _ALLW if w[1] < 1024],                 # pass 0: qb 0-7
    _interleave([w for w in _ALLW if w[1] >= 1024]),   # pass 1: qb 8-15
]


def _pv_flags(order, qb0):
    """Per-pass PV accumulation flags for an arbitrary window order:
    start=True on the first matmul touching each PSUM bank (clears it),
    stop=True on the last matmul into each q-slot."""
    writes = []
    for wi, (c0, a0, b0, a1, b1) in enumerate(order):
        for (c, aa, bb_) in ((c0, a0, b0), (c0 + 1, a1, b1)):
            for qb in range(aa // P, bb_ // P):
                writes.append((wi, c, qb))
    started = set()
    starts, stops = set(), {}
    for (wi, c, qb) in writes:
        bank = 0 if qb - qb0 < 7 else 1
        if bank not in started:
            started.add(bank)
            starts.add((wi, c, qb))
        stops[qb] = (wi, c, qb)
    return starts, set(stops.values())


PV_FLAGS = [_pv_flags(PASS_WINS[p], 8 * p) for p in (0, 1)]
# flattened (head, pass, win) schedule
ITEMS = [(h, p, w) for h in range(HPC) for p in (0, 1) for w in PASS_WINS[p]]


def _build():
    nc = bacc.Bacc("TRN2", target_bir_lowering=False, debug=False,
                   num_devices=NCORES)
    qt_d = nc.dram_tensor("qt", [HPC, D, S], DT.bfloat16, kind="ExternalInput").ap()
    kt_d = nc.dram_tensor("kt", [HPC, D, S], DT.bfloat16, kind="ExternalInput").ap()
    vp_d = nc.dram_tensor("vp", [HPC, P, NCH, DV], DT.bfloat16, kind="ExternalInput").ap()
    bb_d = nc.dram_tensor("bb", [NCH, P, S], DT.bfloat16, kind="ExternalInput").ap()
    ident_d = nc.dram_tensor("ident", [P, P], DT.bfloat16, kind="ExternalInput").ap()
    cw_d = nc.dram_tensor("cw", [P, CW_TOT], DT.float32, kind="ExternalInput").ap()
    out_d = nc.dram_tensor("out", [HPC, P, NCH, D], DT.float32, kind="ExternalOutput").ap()

    with tile.TileContext(nc) as tc:
        with (
            tc.tile_pool(name="cst", bufs=1) as cst_pool,
            tc.tile_pool(name="qk", bufs=3) as qk_pool,
            tc.tile_pool(name="vw", bufs=2) as v_pool,
            tc.tile_pool(name="ex", bufs=3) as ex_pool,
            tc.tile_pool(name="fx", bufs=3) as fx_pool,
            tc.tile_pool(name="fin", bufs=2) as fin_pool,
            tc.tile_pool(name="pss", bufs=3, space="PSUM") as ps_pool,
            tc.tile_pool(name="pso", bufs=1, space="PSUM") as po_pool,
        ):
            ident_t = cst_pool.tile([P, P], DT.bfloat16, tag="ident")
            bb_t = cst_pool.tile([P, NCH, S], DT.bfloat16, tag="bb")
            ct = {}
            for key, off in CW_OFF.items():
                ct[key] = cst_pool.tile([P, 1024], DT.float32,
                                        tag=f"c{key[0]}_{key[1]}",
                                        name=f"c{key[0]}_{key[1]}")

            heads = {}

            def alloc_head(h):
                qt_t = qk_pool.tile([P, S], DT.bfloat16, tag="qt")
                kt_t = qk_pool.tile([P, S], DT.bfloat16, tag="kt")
                v_t = v_pool.tile([P, NCH, DV], DT.bfloat16, tag="vp")
                dma = nc.sync.dma_start

                def qk_span(x, y):
                    dma(qt_t[0:D, x:y], qt_d[h][:, x:y])
                    dma(qt_t[D:P, x:y], qt_d[h][:, x:y])
                    dma(kt_t[0:D, x:y], kt_d[h][:, x:y])
                    dma(kt_t[D:P, x:y], kt_d[h][:, x:y])

                if h == 0:
                    # boot: land exactly what the first windows need, then
                    # stream the bias/C tables behind the early compute
                    qk_span(0, 512)
                    dma(ident_t[:], ident_d[:])
                    for c in (0, 1):
                        dma(bb_t[:, c, :], bb_d[c])
                    qk_span(512, 1024)
                    for c in (2, 3):
                        dma(bb_t[:, c, :], bb_d[c])
                    dma(v_t[:], vp_d[h])
                    qk_span(1024, S)
                    for c in (4, 5, 6, 7):
                        dma(bb_t[:, c, :], bb_d[c])
                    for key in DVE_WINS[:4]:
                        dma(ct[key][:], cw_d[:, CW_OFF[key]:CW_OFF[key] + 1024])
                    for c in (8, 9, 10, 11):
                        dma(bb_t[:, c, :], bb_d[c])
                    for key in DVE_WINS[4:]:
                        dma(ct[key][:], cw_d[:, CW_OFF[key]:CW_OFF[key] + 1024])
                    for c in (12, 13, 14, 15):
                        dma(bb_t[:, c, :], bb_d[c])
                else:
                    qk_span(0, S)
                    dma(v_t[:], vp_d[h])
                heads[h] = (qt_t, kt_t, v_t)

            live_ps = {}

            def emit_qk(h, win):
                (c0, a0, b0, a1, b1) = win
                qt_t, kt_t, _ = heads[h]
                u0, u1 = b0 - a0, b1 - a1
                g0 = 512 - u0
                act = (c0, a0) not in DVE_SET
                ps = ps_pool.tile([P, 1024], DT.float32, tag="st")
                # bias matmuls are emitted separately (emit_bias), after
                # some PV work of an older window fills the drain bubble
                # chunk c0 -> [g0, 512) (bank 0) from PE rows 0-63; chunk
                # c1 -> [512, 512+u1) (bank 1) from rows 64-127; for ACT
                # windows the fp8 DoubleRow bias matmuls then accumulate
                # 184.66*bias^T into each bank (with the causal mask baked
                # into b8 at the diagonal blocks)
                nc.tensor.matmul(
                    ps[:, g0:512],
                    kt_t[0:D, P * c0:P * (c0 + 1)],
                    qt_t[0:D, a0:b0],
                    start=True, stop=not act,
                )
                nc.tensor.matmul(
                    ps[:, 512:512 + u1],
                    kt_t[D:P, P * (c0 + 1):P * (c0 + 2)],
                    qt_t[D:P, a1:b1],
                    start=True, stop=not act,
                )
                live_ps[(h, c0, a0)] = ps

            def emit_bias(h, win):
                (c0, a0, b0, a1, b1) = win
                if (c0, a0) in DVE_SET:
                    return
                u1 = b1 - a1
                g0 = 512 - (b0 - a0)
                ps = live_ps[(h, c0, a0)]
                nc.tensor.matmul(
                    ps[:, g0:512], ident_t[:], bb_t[:, c0, a0:b0],
                    start=False, stop=True, skip_group_check=True,
                )
                nc.tensor.matmul(
                    ps[:, 512:512 + u1],
                    ident_t[:], bb_t[:, c0 + 1, a1:b1],
                    start=False, stop=True, skip_group_check=True,
                )

            accs = {}

            alloc_head(0)
            emit_qk(ITEMS[0][0], ITEMS[0][2])
            emit_bias(ITEMS[0][0], ITEMS[0][2])
            emit_qk(ITEMS[1][0], ITEMS[1][2])
            emit_bias(ITEMS[1][0], ITEMS[1][2])
            for idx, (h, pss, win) in enumerate(ITEMS):
                first_of_pass = idx == 0 or ITEMS[idx - 1][:2] != (h, pss)
                last_of_pass = (idx + 1 == len(ITEMS)
                                or ITEMS[idx + 1][:2] != (h, pss))
                if first_of_pass:
                    if pss == 0 and h + 1 < HPC:
                        alloc_head(h + 1)
                    # per-pass PV accumulators: 8 slots of [128, 65]
                    # packed 7 + 1 in 2 PSUM banks
                    oa = po_pool.tile([P, 7, DV], DT.float32, tag="oa")
                    ob = po_pool.tile([P, 1, DV], DT.float32, tag="ob")
                    accs[(h, pss)] = (oa, ob)
                nxt = None
                if idx + 2 < len(ITEMS):
                    nh, _, nwin = ITEMS[idx + 2]
                    emit_qk(nh, nwin)
                    nxt = (nh, nwin)

                (c0, a0, b0, a1, b1) = win
                key = (c0, a0)
                u0, u1 = b0 - a0, b1 - a1
                g0 = 512 - u0
                w = 512 + u1
                ps = live_ps.pop((h, c0, a0))

                if key in DVE_SET:
                    it = fx_pool.tile([P, 1024], DT.int16, tag="fx")
                    nc.vector.tensor_tensor(
                        it[:, g0:w], ps[:, g0:w],
                        ct[key][:, 0:w - g0], mybir.AluOpType.add,
                    )
                    pts = it[:].bitcast(DT.bfloat16)
                else:
                    ex = ex_pool.tile([P, 1024], DT.bfloat16, tag="ex")
                    nc.scalar.activation(
                        ex[:, g0:w], ps[:, g0:w], AF.Exp, scale=ACT_SCALE)
                    pts = ex[:]

                oa, ob = accs[(h, pss)]
                qb0 = 8 * pss

                def oslot(qb):
                    r = qb - qb0
                    return oa[:, r, :] if r < 7 else ob[:, 0, :]

                v_t = heads[h][2]
                starts, stops = PV_FLAGS[pss]
                wi = PASS_WINS[pss].index(win)
                halves = ((c0, a0, b0, g0), (c0 + 1, a1, b1, 512))
                for hi, (c, aa, bb_, toff) in enumerate(halves):
                    for qb in range(aa // P, bb_ // P):
                        nc.tensor.matmul(
                            oslot(qb),
                            pts[:, toff + qb * P - aa:
                                toff + qb * P - aa + P],
                            v_t[:, c, :],
                            start=(wi, c, qb) in starts,
                            stop=(wi, c, qb) in stops,
                            skip_group_check=True,
                        )
                    if hi == 0 and nxt is not None:
                        # window i+2's bias matmuls slot in here, after
                        # the first PV half has covered their QK's
                        # accumulate-drain latency
                        emit_bias(*nxt)
                        nxt = None
                if nxt is not None:
                    emit_bias(*nxt)

                if last_of_pass:
                    oa, ob = accs.pop((h, pss))
                    rec = fin_pool.tile([P, 8], DT.float32, tag="rec")
                    nc.vector.reciprocal(rec[:, 0:7], oa[:, :, D])
                    nc.vector.reciprocal(rec[:, 7:8], ob[:, :, D])
                    outf = fin_pool.tile([P, 8, D], DT.float32, tag="outf")
                    for (acc, lo, hi) in ((oa, 0, 7), (ob, 7, 8)):
                        a, bb = bass.broadcast_tensor_aps(
                            acc[:, 0:hi - lo, 0:D],
                            rec[:, lo:hi].rearrange("p (n o) -> p n o", o=1),
                        )
                        nc.vector.tensor_tensor(
                            outf[:, lo:hi, :], a, bb, mybir.AluOpType.mult)
                    nc.sync.dma_start(
                        out_d[h][:, 8 * pss:8 * pss + 8, :], outf[:])
                    if pss == 1:
                        heads.pop(h)

    nc.finalize()
    return nc


def kernel(queries, keys, values, queries_mask, values_mask, bias):
    global _built, LAST_EXEC_NS
    q = np.asarray(queries, dtype=np.float32)
    k = np.asarray(keys, dtype=np.float32)
    v = np.asarray(values, dtype=np.float32)
    bias = np.asarray(bias, dtype=np.float32)

    qT = np.ascontiguousarray(
        (q * ALPHA).transpose(0, 1, 3, 2)).astype(ml_dtypes.bfloat16)
    kT = np.ascontiguousarray(
        k.transpose(0, 1, 3, 2)).astype(ml_dtypes.bfloat16)  # [B,H,D,S]
    vp = np.ones((B, H, S, DV), dtype=ml_dtypes.bfloat16)
    vp[..., :D] = v.astype(ml_dtypes.bfloat16)
    # [B,H,P,NCH,DV] so the device DMA is fully contiguous
    vp = np.ascontiguousarray(
        vp.reshape(B, H, NCH, P, DV).transpose(0, 1, 3, 2, 4))

    btf = np.ascontiguousarray(bias[:, 0].transpose(0, 2, 1))  # [B,S,S] (k,q)
    # bias-add moving operand [B, NCH, P, S]: CB_MUL*biasT in bf16, with
    # the upper triangle (k > q) of each diagonal block masked to -40000
    # (exp underflows to exactly 0)
    bb = (CB_MUL * btf).reshape(B, NCH, P, S)
    m = np.arange(P)[:, None] > np.arange(P)[None, :]   # in-chunk k > q
    for c in range(NCH):
        blk = bb[:, c, :, P * c:P * (c + 1)]
        blk[np.broadcast_to(m[None], blk.shape)] = -40000.0
    bb = bb.astype(ml_dtypes.bfloat16)
    ident = np.eye(P, dtype=ml_dtypes.bfloat16)
    # host-precomputed Schraudolph bias bits for the DVE windows
    cw = np.empty((B, P, CW_TOT), dtype=np.float32)
    for (c0, a0), off in CW_OFF.items():
        for half in (0, 1):
            rows = slice(P * (c0 + half), P * (c0 + half + 1))
            cols = slice(off + 512 * half, off + 512 * (half + 1))
            cw[:, :, cols] = CB_MUL * btf[:, rows, a0:a0 + 512] + CB_ADD

    if _built is None:
        _built = _build()
    nc = _built

    in_maps = []
    for c in range(NCORES):
        b, h0 = c // 2, (c % 2) * HPC
        in_maps.append({
            "qt": np.ascontiguousarray(qT[b, h0:h0 + HPC]),
            "kt": np.ascontiguousarray(kT[b, h0:h0 + HPC]),
            "vp": np.ascontiguousarray(vp[b, h0:h0 + HPC]),
            "bb": bb[b],
            "ident": ident,
            "cw": cw[b],
        })

    global LAST_PROFILE_DIR
    if TRACE:
        res, LAST_EXEC_NS, LAST_PROFILE_DIR = _nrt_profile_run(nc, in_maps)
    else:
        res = run_bass_kernel_spmd(nc, in_maps, core_ids=list(range(NCORES)))
        LAST_EXEC_NS = None

    out = np.empty((B, H, S, D), dtype=np.float32)
    for c in range(NCORES):
        b, h0 = c // 2, (c % 2) * HPC
        r = np.asarray(res.results[c]["out"])
        out[b, h0:h0 + HPC] = r.transpose(0, 2, 1, 3).reshape(HPC, S, D)
    return out


# revision 13
# speedup vs baseline: 1.1985x; 1.1985x over previous
"""Distributed causal-attention-with-bias Bass kernel for 8 TRN2 NeuronCores.

Problem (hardcoded): B=4, H=16, S=2048, D=64
  out = softmax(Q K^T / sqrt(D) + bias, causal) @ V

Sharding: core c handles batch b = c//2, heads h in [8*(c%2), 8*(c%2)+8).
Per-(b,h) attention is fully independent; bias[b] is shared by the 8 heads
on a core.

Per core, per head h, window of a k-chunk pair (c0, c0+1):
  psum[k,q]  = K_c @ (ALPHA*Q)^T       (TensorE bf16, paired row groups;
                                        psum lands in Schraudolph units)
  psum[k,q] += 184.66*bias^T[k,q]      (TensorE fp8 DoubleRow: 16*I
                                        stationary x 11.54*bias^T moving;
                                        causal mask baked into the fp8
                                        bias at diagonal blocks.  ACT
                                        windows only)
  ACT windows:  P^T = exp(psum*ln2/128)           (ScalarE, one op)
  DVE windows:  P^T = bitcast(int16(psum + C))    (VectorE, one op;
                                        C = 184.66*bias^T + const from
                                        host; Schraudolph bf16 exp)
  out[q,d+1] += P^T_slice^T @ [V_c|1]  (TensorE; ones col -> denominator)
  out[q,:]    = acc[q,0:64]/acc[q,64]  (VectorE recip+mul from PSUM)

Scheduling: q-blocks processed in 2 passes of 8 (accumulators fit 2 PSUM
banks/pass), freeing 6 banks for 3 score windows in flight; the QK+bias
matmuls of window i+2 are issued to the PE queue BEFORE the PV matmuls of
window i, so the ~2us cross-engine exp chain hides behind PE streaming
and the PE never idles (idle gaps let the HAM clock-gate halve the PE
clock).
"""

import sys

if "/opt/trn_rl_repo" not in sys.path:
    sys.path.insert(0, "/opt/trn_rl_repo")

import math

import ml_dtypes
import numpy as np

import concourse.bass as bass
import concourse.tile as tile
from concourse import bacc, mybir
from concourse.bass_utils import run_bass_kernel_spmd

DT = mybir.dt
AF = mybir.ActivationFunctionType
FP8 = ml_dtypes.float8_e4m3

B, H, S, D = 4, 16, 2048, 64
P = 128              # partition dim / k-chunk size
NCH = S // P         # 16 k-chunks
HPC = H // 2         # 8 heads per core
NCORES = 8
DV = D + 1           # V padded with a ones column

TRACE = False
LAST_EXEC_NS = None
LAST_PROFILE_DIR = None

# Schraudolph constants: with Q prescaled by ALPHA on the host, the QK psum
# is directly in bf16-bit units; adding C = CB_MUL*bias^T + CB_ADD and
# converting to int16 gives the bf16 bit pattern of exp(QK/8 + bias).
ALPHA = 16.0 / math.log(2.0)
ACT_SCALE = math.log(2.0) / 128.0          # psum*ACT_SCALE = QK/8 + bias
CB_MUL = 128.0 / math.log(2.0)             # = 184.66 = 16 * 11.5415
CB_ADD = 128.0 * (127.0 - 0.0579)
BIAS_STAT = 16.0                           # fp8 identity stationary value
BIAS_MOV = CB_MUL / BIAS_STAT              # fp8 bias moving scale
MASK8 = -240.0                             # e4m3 max-negative for causal

# Windows (keyed by (pair_c0, window_start_q)) routed to the fused
# Schraudolph path on VectorE instead of ScalarE exp.  Only windows with
# q >= 1024 (large effective key count -> approximation error averages
# out) and no diagonal blocks are eligible; all have u0 = u1 = 512.
DVE_WINS = ((0, 512), (0, 1024), (0, 1536), (2, 512), (2, 1024),
            (2, 1536), (4, 1024), (4, 1536))
DVE_SET = set(DVE_WINS)
CW_OFF = {k: 1024 * i for i, k in enumerate(DVE_WINS)}
CW_TOT = 1024 * len(DVE_WINS)

_built = None


def _nrt_profile_run(nc, in_maps):
    """Run via SPMD with the axon NRT profiler capturing NTFFs, then parse
    core 0's NTFF with neuron-profile to get the NEFF exec time in ns."""
    import ctypes
    import tempfile

    lib = ctypes.CDLL("/opt/axon/libaxon_pjrt.so")
    for f in (lib.axon_start_nrt_profile, lib.axon_stop_nrt_profile):
        f.restype = ctypes.c_int64
        f.argtypes = [ctypes.c_char_p, ctypes.c_size_t]
    d = tempfile.mkdtemp(prefix="attnprof_")
    b = d.encode()
    assert lib.axon_start_nrt_profile(b, len(b)) == 0
    try:
        res = run_bass_kernel_spmd(nc, in_maps, core_ids=list(range(NCORES)))
    finally:
        lib.axon_stop_nrt_profile(b, len(b))
    exec_ns = None
    try:
        from gauge.profiler import FishPath, Profile
        prof = Profile(
            profile_path=FishPath(d), kernel_dev_mode=True,
            profile_on_exit=False, bass_kernel=nc.m,
            offline_processing=True, fname="*_body*",
        )
        prof.convert_ntffs_to_json((0,))
        exec_ns = int(prof.get_total_time(0) * 1e9)
    except Exception as e:  # profiling is best-effort
        print(f"ntff parse failed: {e!r}")
    return res, exec_ns, d


def _pair_windows(c0):
    qs0, qs1 = P * c0, P * (c0 + 1)
    out = []
    for j in range(qs0 // 512, S // 512):
        a0, b0 = max(qs0, 512 * j), 512 * (j + 1)
        a1, b1 = max(qs1, 512 * j), 512 * (j + 1)
        out.append((c0, a0, b0, a1, b1))
    return out


_ALLW = [w for c0 in range(0, NCH, 2) for w in _pair_windows(c0)]


def _interleave(wins):
    """Alternate DVE- and ACT-path windows so the two exp engines take
    turns producing P^T and neither becomes the lone producer (which
    starves the PE and lets the HAM clock-gate cool it)."""
    dve = [w for w in wins if (w[0], w[1]) in DVE_SET]
    act = [w for w in wins if (w[0], w[1]) not in DVE_SET]
    out = []
    while dve or act:
        if dve:
            out.append(dve.pop(0))
        if act:
            out.append(act.pop(0))
    return out


PASS_WINS = [
    _interleave([w for w in _ALLW if w[1] < 1024]),    # pass 0: qb 0-7
    _interleave([w for w in _ALLW if w[1] >= 1024]),   # pass 1: qb 8-15
]


def _pv_flags(order, qb0):
    """Per-pass PV accumulation flags for an arbitrary window order:
    start=True on the first matmul touching each PSUM bank (clears it),
    stop=True on the last matmul into each q-slot."""
    writes = []
    for wi, (c0, a0, b0, a1, b1) in enumerate(order):
        for (c, aa, bb_) in ((c0, a0, b0), (c0 + 1, a1, b1)):
            for qb in range(aa // P, bb_ // P):
                writes.append((wi, c, qb))
    started = set()
    starts, stops = set(), {}
    for (wi, c, qb) in writes:
        bank = 0 if qb - qb0 < 7 else 1
        if bank not in started:
            started.add(bank)
            starts.add((wi, c, qb))
        stops[qb] = (wi, c, qb)
    return starts, set(stops.values())


PV_FLAGS = [_pv_flags(PASS_WINS[p], 8 * p) for p in (0, 1)]
# flattened (head, pass, win) schedule
ITEMS = [(h, p, w) for h in range(HPC) for p in (0, 1) for w in PASS_WINS[p]]


def _build():
    nc = bacc.Bacc("TRN2", target_bir_lowering=False, debug=False,
                   num_devices=NCORES)
    qt_d = nc.dram_tensor("qt", [HPC, D, S], DT.bfloat16, kind="ExternalInput").ap()
    kt_d = nc.dram_tensor("kt", [HPC, D, S], DT.bfloat16, kind="ExternalInput").ap()
    vp_d = nc.dram_tensor("vp", [HPC, P, NCH, DV], DT.bfloat16, kind="ExternalInput").ap()
    bb_d = nc.dram_tensor("bb", [NCH, P, S], DT.bfloat16, kind="ExternalInput").ap()
    ident_d = nc.dram_tensor("ident", [P, P], DT.bfloat16, kind="ExternalInput").ap()
    cw_d = nc.dram_tensor("cw", [P, CW_TOT], DT.float32, kind="ExternalInput").ap()
    out_d = nc.dram_tensor("out", [HPC, P, NCH, D], DT.float32, kind="ExternalOutput").ap()

    with tile.TileContext(nc) as tc:
        with (
            tc.tile_pool(name="cst", bufs=1) as cst_pool,
            tc.tile_pool(name="qk", bufs=3) as qk_pool,
            tc.tile_pool(name="vw", bufs=2) as v_pool,
            tc.tile_pool(name="ex", bufs=3) as ex_pool,
            tc.tile_pool(name="fx", bufs=3) as fx_pool,
            tc.tile_pool(name="fin", bufs=2) as fin_pool,
            tc.tile_pool(name="pss", bufs=3, space="PSUM") as ps_pool,
            tc.tile_pool(name="pso", bufs=1, space="PSUM") as po_pool,
        ):
            ident_t = cst_pool.tile([P, P], DT.bfloat16, tag="ident")
            bb_t = cst_pool.tile([P, NCH, S], DT.bfloat16, tag="bb")
            ct = {}
            for key, off in CW_OFF.items():
                ct[key] = cst_pool.tile([P, 1024], DT.float32,
                                        tag=f"c{key[0]}_{key[1]}",
                                        name=f"c{key[0]}_{key[1]}")

            heads = {}

            def alloc_head(h):
                qt_t = qk_pool.tile([P, S], DT.bfloat16, tag="qt")
                kt_t = qk_pool.tile([P, S], DT.bfloat16, tag="kt")
                v_t = v_pool.tile([P, NCH, DV], DT.bfloat16, tag="vp")
                dma = nc.sync.dma_start

                def qk_span(x, y):
                    dma(qt_t[0:D, x:y], qt_d[h][:, x:y])
                    dma(qt_t[D:P, x:y], qt_d[h][:, x:y])
                    dma(kt_t[0:D, x:y], kt_d[h][:, x:y])
                    dma(kt_t[D:P, x:y], kt_d[h][:, x:y])

                if h == 0:
                    # boot: land exactly what the first windows need; the
                    # scalar engine's hardware DGE issues the bias chunks
                    # in parallel with the sync queue
                    qk_span(0, 512)
                    nc.scalar.dma_start(ident_t[:], ident_d[:])
                    for c in (0, 1, 2, 3):
                        nc.scalar.dma_start(bb_t[:, c, :], bb_d[c])
                    qk_span(512, 1024)
                    dma(v_t[:], vp_d[h])
                    qk_span(1024, S)
                    for c in (4, 5, 6, 7):
                        dma(bb_t[:, c, :], bb_d[c])
                    for key in DVE_WINS[:4]:
                        dma(ct[key][:], cw_d[:, CW_OFF[key]:CW_OFF[key] + 1024])
                    for c in (8, 9, 10, 11):
                        dma(bb_t[:, c, :], bb_d[c])
                    for key in DVE_WINS[4:]:
                        dma(ct[key][:], cw_d[:, CW_OFF[key]:CW_OFF[key] + 1024])
                    for c in (12, 13, 14, 15):
                        dma(bb_t[:, c, :], bb_d[c])
                else:
                    qk_span(0, S)
                    dma(v_t[:], vp_d[h])
                heads[h] = (qt_t, kt_t, v_t)

            live_ps = {}

            def emit_qk(h, win):
                (c0, a0, b0, a1, b1) = win
                qt_t, kt_t, _ = heads[h]
                u0, u1 = b0 - a0, b1 - a1
                g0 = 512 - u0
                act = (c0, a0) not in DVE_SET
                ps = ps_pool.tile([P, 1024], DT.float32, tag="st")
                # chunk c0 -> [g0, 512) (bank 0) from PE rows 0-63; chunk
                # c1 -> [512, 512+u1) (bank 1) from rows 64-127; for ACT
                # windows the fp8 DoubleRow bias matmuls then accumulate
                # 184.66*bias^T into each bank (with the causal mask baked
                # into b8 at the diagonal blocks)
                nc.tensor.matmul(
                    ps[:, g0:512],
                    kt_t[0:D, P * c0:P * (c0 + 1)],
                    qt_t[0:D, a0:b0],
                    start=True, stop=not act,
                )
                nc.tensor.matmul(
                    ps[:, 512:512 + u1],
                    kt_t[D:P, P * (c0 + 1):P * (c0 + 2)],
                    qt_t[D:P, a1:b1],
                    start=True, stop=not act,
                )
                if act:
                    nc.tensor.matmul(
                        ps[:, g0:512], ident_t[:], bb_t[:, c0, a0:b0],
                        start=False, stop=True, skip_group_check=True,
                    )
                    nc.tensor.matmul(
                        ps[:, 512:512 + u1],
                        ident_t[:], bb_t[:, c0 + 1, a1:b1],
                        start=False, stop=True, skip_group_check=True,
                    )
                live_ps[(h, c0, a0)] = ps

            accs = {}

            alloc_head(0)
            emit_qk(*(lambda it: (it[0], it[2]))(ITEMS[0]))
            emit_qk(*(lambda it: (it[0], it[2]))(ITEMS[1]))
            for idx, (h, pss, win) in enumerate(ITEMS):
                first_of_pass = idx == 0 or ITEMS[idx - 1][:2] != (h, pss)
                last_of_pass = (idx + 1 == len(ITEMS)
                                or ITEMS[idx + 1][:2] != (h, pss))
                if first_of_pass:
                    if pss == 0 and h + 1 < HPC:
                        alloc_head(h + 1)
                    # per-pass PV accumulators: 8 slots of [128, 65]
                    # packed 7 + 1 in 2 PSUM banks
                    oa = po_pool.tile([P, 7, DV], DT.float32, tag="oa")
                    ob = po_pool.tile([P, 1, DV], DT.float32, tag="ob")
                    accs[(h, pss)] = (oa, ob)
                if idx + 2 < len(ITEMS):
                    nh, _, nwin = ITEMS[idx + 2]
                    emit_qk(nh, nwin)

                (c0, a0, b0, a1, b1) = win
                key = (c0, a0)
                u0, u1 = b0 - a0, b1 - a1
                g0 = 512 - u0
                w = 512 + u1
                ps = live_ps.pop((h, c0, a0))

                if key in DVE_SET:
                    it = fx_pool.tile([P, 1024], DT.int16, tag="fx")
                    nc.vector.tensor_tensor(
                        it[:, g0:w], ps[:, g0:w],
                        ct[key][:, 0:w - g0], mybir.AluOpType.add,
                    )
                    pts = it[:].bitcast(DT.bfloat16)
                else:
                    ex = ex_pool.tile([P, 1024], DT.bfloat16, tag="ex")
                    nc.scalar.activation(
                        ex[:, g0:w], ps[:, g0:w], AF.Exp, scale=ACT_SCALE)
                    pts = ex[:]

                oa, ob = accs[(h, pss)]
                qb0 = 8 * pss

                def oslot(qb):
                    r = qb - qb0
                    return oa[:, r, :] if r < 7 else ob[:, 0, :]

                v_t = heads[h][2]
                starts, stops = PV_FLAGS[pss]
                wi = PASS_WINS[pss].index(win)
                for (c, aa, bb_, toff) in ((c0, a0, b0, g0),
                                           (c0 + 1, a1, b1, 512)):
                    for qb in range(aa // P, bb_ // P):
                        nc.tensor.matmul(
                            oslot(qb),
                            pts[:, toff + qb * P - aa:
                                toff + qb * P - aa + P],
                            v_t[:, c, :],
                            start=(wi, c, qb) in starts,
                            stop=(wi, c, qb) in stops,
                            skip_group_check=True,
                        )

                if last_of_pass:
                    oa, ob = accs.pop((h, pss))
                    rec = fin_pool.tile([P, 8], DT.float32, tag="rec")
                    nc.vector.reciprocal(rec[:, 0:7], oa[:, :, D])
                    nc.vector.reciprocal(rec[:, 7:8], ob[:, :, D])
                    outf = fin_pool.tile([P, 8, D], DT.float32, tag="outf")
                    for (acc, lo, hi) in ((oa, 0, 7), (ob, 7, 8)):
                        a, bb = bass.broadcast_tensor_aps(
                            acc[:, 0:hi - lo, 0:D],
                            rec[:, lo:hi].rearrange("p (n o) -> p n o", o=1),
                        )
                        nc.vector.tensor_tensor(
                            outf[:, lo:hi, :], a, bb, mybir.AluOpType.mult)
                    nc.sync.dma_start(
                        out_d[h][:, 8 * pss:8 * pss + 8, :], outf[:])
                    if pss == 1:
                        heads.pop(h)

    nc.finalize()
    return nc


def kernel(queries, keys, values, queries_mask, values_mask, bias):
    global _built, LAST_EXEC_NS
    q = np.asarray(queries, dtype=np.float32)
    k = np.asarray(keys, dtype=np.float32)
    v = np.asarray(values, dtype=np.float32)
    bias = np.asarray(bias, dtype=np.float32)

    qT = np.ascontiguousarray(
        (q * ALPHA).transpose(0, 1, 3, 2)).astype(ml_dtypes.bfloat16)
    kT = np.ascontiguousarray(
        k.transpose(0, 1, 3, 2)).astype(ml_dtypes.bfloat16)  # [B,H,D,S]
    vp = np.ones((B, H, S, DV), dtype=ml_dtypes.bfloat16)
    vp[..., :D] = v.astype(ml_dtypes.bfloat16)
    # [B,H,P,NCH,DV] so the device DMA is fully contiguous
    vp = np.ascontiguousarray(
        vp.reshape(B, H, NCH, P, DV).transpose(0, 1, 3, 2, 4))

    btf = np.ascontiguousarray(bias[:, 0].transpose(0, 2, 1))  # [B,S,S] (k,q)
    # bias-add moving operand [B, NCH, P, S]: CB_MUL*biasT in bf16, with
    # the upper triangle (k > q) of each diagonal block masked to -40000
    # (exp underflows to exactly 0)
    bb = (CB_MUL * btf).reshape(B, NCH, P, S)
    m = np.arange(P)[:, None] > np.arange(P)[None, :]   # in-chunk k > q
    for c in range(NCH):
        blk = bb[:, c, :, P * c:P * (c + 1)]
        blk[np.broadcast_to(m[None], blk.shape)] = -40000.0
    bb = bb.astype(ml_dtypes.bfloat16)
    ident = np.eye(P, dtype=ml_dtypes.bfloat16)
    # host-precomputed Schraudolph bias bits for the DVE windows
    cw = np.empty((B, P, CW_TOT), dtype=np.float32)
    for (c0, a0), off in CW_OFF.items():
        for half in (0, 1):
            rows = slice(P * (c0 + half), P * (c0 + half + 1))
            cols = slice(off + 512 * half, off + 512 * (half + 1))
            cw[:, :, cols] = CB_MUL * btf[:, rows, a0:a0 + 512] + CB_ADD

    if _built is None:
        _built = _build()
    nc = _built

    in_maps = []
    for c in range(NCORES):
        b, h0 = c // 2, (c % 2) * HPC
        in_maps.append({
            "qt": np.ascontiguousarray(qT[b, h0:h0 + HPC]),
            "kt": np.ascontiguousarray(kT[b, h0:h0 + HPC]),
            "vp": np.ascontiguousarray(vp[b, h0:h0 + HPC]),
            "bb": bb[b],
            "ident": ident,
            "cw": cw[b],
        })

    global LAST_PROFILE_DIR
    if TRACE:
        res, LAST_EXEC_NS, LAST_PROFILE_DIR = _nrt_profile_run(nc, in_maps)
    else:
        res = run_bass_kernel_spmd(nc, in_maps, core_ids=list(range(NCORES)))
        LAST_EXEC_NS = None

    out = np.empty((B, H, S, D), dtype=np.float32)
    for c in range(NCORES):
        b, h0 = c // 2, (c % 2) * HPC
        r = np.asarray(res.results[c]["out"])
        out[b, h0:h0 + HPC] = r.transpose(0, 2, 1, 3).reshape(HPC, S, D)
    return out


# revision 14
# speedup vs baseline: 1.2134x; 1.0124x over previous
"""Distributed causal-attention-with-bias Bass kernel for 8 TRN2 NeuronCores.

Problem (hardcoded): B=4, H=16, S=2048, D=64
  out = softmax(Q K^T / sqrt(D) + bias, causal) @ V

Sharding: core c handles batch b = c//2, heads h in [8*(c%2), 8*(c%2)+8).
Per-(b,h) attention is fully independent; bias[b] is shared by the 8 heads
on a core.

Per core, per head h, window of a k-chunk pair (c0, c0+1):
  psum[k,q]  = K_c @ (ALPHA*Q)^T       (TensorE bf16, paired row groups;
                                        psum lands in Schraudolph units)
  psum[k,q] += 184.66*bias^T[k,q]      (TensorE fp8 DoubleRow: 16*I
                                        stationary x 11.54*bias^T moving;
                                        causal mask baked into the fp8
                                        bias at diagonal blocks.  ACT
                                        windows only)
  ACT windows:  P^T = exp(psum*ln2/128)           (ScalarE, one op)
  DVE windows:  P^T = bitcast(int16(psum + C))    (VectorE, one op;
                                        C = 184.66*bias^T + const from
                                        host; Schraudolph bf16 exp)
  out[q,d+1] += P^T_slice^T @ [V_c|1]  (TensorE; ones col -> denominator)
  out[q,:]    = acc[q,0:64]/acc[q,64]  (VectorE recip+mul from PSUM)

Scheduling: q-blocks processed in 2 passes of 8 (accumulators fit 2 PSUM
banks/pass), freeing 6 banks for 3 score windows in flight; the QK+bias
matmuls of window i+2 are issued to the PE queue BEFORE the PV matmuls of
window i, so the ~2us cross-engine exp chain hides behind PE streaming
and the PE never idles (idle gaps let the HAM clock-gate halve the PE
clock).
"""

import sys

if "/opt/trn_rl_repo" not in sys.path:
    sys.path.insert(0, "/opt/trn_rl_repo")

import math

import ml_dtypes
import numpy as np

import concourse.bass as bass
import concourse.tile as tile
from concourse import bacc, mybir
from concourse.bass_utils import run_bass_kernel_spmd

DT = mybir.dt
AF = mybir.ActivationFunctionType
FP8 = ml_dtypes.float8_e4m3

B, H, S, D = 4, 16, 2048, 64
P = 128              # partition dim / k-chunk size
NCH = S // P         # 16 k-chunks
HPC = H // 2         # 8 heads per core
NCORES = 8
DV = D + 1           # V padded with a ones column

TRACE = False
LAST_EXEC_NS = None
LAST_PROFILE_DIR = None

# Schraudolph constants: with Q prescaled by ALPHA on the host, the QK psum
# is directly in bf16-bit units; adding C = CB_MUL*bias^T + CB_ADD and
# converting to int16 gives the bf16 bit pattern of exp(QK/8 + bias).
ALPHA = 16.0 / math.log(2.0)
ACT_SCALE = math.log(2.0) / 128.0          # psum*ACT_SCALE = QK/8 + bias
CB_MUL = 128.0 / math.log(2.0)             # = 184.66 = 16 * 11.5415
CB_ADD = 128.0 * (127.0 - 0.0579)
BIAS_STAT = 16.0                           # fp8 identity stationary value
BIAS_MOV = CB_MUL / BIAS_STAT              # fp8 bias moving scale
MASK8 = -240.0                             # e4m3 max-negative for causal

# Windows (keyed by (pair_c0, window_start_q)) routed to the fused
# Schraudolph path on VectorE instead of ScalarE exp.  Only windows with
# q >= 1024 (large effective key count -> approximation error averages
# out) and no diagonal blocks are eligible; all have u0 = u1 = 512.
DVE_WINS = ((0, 512), (0, 1024), (0, 1536), (2, 512), (2, 1024),
            (2, 1536), (4, 1024), (4, 1536))
DVE_SET = set(DVE_WINS)
CW_OFF = {k: 1024 * i for i, k in enumerate(DVE_WINS)}
CW_TOT = 1024 * len(DVE_WINS)

_built = None


def _nrt_profile_run(nc, in_maps):
    """Run via SPMD with the axon NRT profiler capturing NTFFs, then parse
    core 0's NTFF with neuron-profile to get the NEFF exec time in ns."""
    import ctypes
    import tempfile

    lib = ctypes.CDLL("/opt/axon/libaxon_pjrt.so")
    for f in (lib.axon_start_nrt_profile, lib.axon_stop_nrt_profile):
        f.restype = ctypes.c_int64
        f.argtypes = [ctypes.c_char_p, ctypes.c_size_t]
    d = tempfile.mkdtemp(prefix="attnprof_")
    b = d.encode()
    assert lib.axon_start_nrt_profile(b, len(b)) == 0
    try:
        res = run_bass_kernel_spmd(nc, in_maps, core_ids=list(range(NCORES)))
    finally:
        lib.axon_stop_nrt_profile(b, len(b))
    exec_ns = None
    try:
        from gauge.profiler import FishPath, Profile
        prof = Profile(
            profile_path=FishPath(d), kernel_dev_mode=True,
            profile_on_exit=False, bass_kernel=nc.m,
            offline_processing=True, fname="*_body*",
        )
        prof.convert_ntffs_to_json((0,))
        exec_ns = int(prof.get_total_time(0) * 1e9)
    except Exception as e:  # profiling is best-effort
        print(f"ntff parse failed: {e!r}")
    return res, exec_ns, d


def _pair_windows(c0):
    qs0, qs1 = P * c0, P * (c0 + 1)
    out = []
    for j in range(qs0 // 512, S // 512):
        a0, b0 = max(qs0, 512 * j), 512 * (j + 1)
        a1, b1 = max(qs1, 512 * j), 512 * (j + 1)
        out.append((c0, a0, b0, a1, b1))
    return out


_ALLW = [w for c0 in range(0, NCH, 2) for w in _pair_windows(c0)]


def _interleave(wins):
    """Alternate DVE- and ACT-path windows so the two exp engines take
    turns producing P^T and neither becomes the lone producer (which
    starves the PE and lets the HAM clock-gate cool it)."""
    dve = [w for w in wins if (w[0], w[1]) in DVE_SET]
    act = [w for w in wins if (w[0], w[1]) not in DVE_SET]
    out = []
    while dve or act:
        if dve:
            out.append(dve.pop(0))
        if act:
            out.append(act.pop(0))
    return out


PASS_WINS = [
    _interleave([w for w in _ALLW if w[1] < 1024]),    # pass 0: qb 0-7
    _interleave([w for w in _ALLW if w[1] >= 1024]),   # pass 1: qb 8-15
]


def _pv_flags(order, qb0):
    """Per-pass PV accumulation flags for an arbitrary window order:
    start=True on the first matmul touching each PSUM bank (clears it),
    stop=True on the last matmul into each q-slot."""
    writes = []
    for wi, (c0, a0, b0, a1, b1) in enumerate(order):
        for (c, aa, bb_) in ((c0, a0, b0), (c0 + 1, a1, b1)):
            for qb in range(aa // P, bb_ // P):
                writes.append((wi, c, qb))
    started = set()
    starts, stops = set(), {}
    for (wi, c, qb) in writes:
        bank = 0 if qb - qb0 < 7 else 1
        if bank not in started:
            started.add(bank)
            starts.add((wi, c, qb))
        stops[qb] = (wi, c, qb)
    return starts, set(stops.values())


PV_FLAGS = [_pv_flags(PASS_WINS[p], 8 * p) for p in (0, 1)]
# flattened (head, pass, win) schedule
ITEMS = [(h, p, w) for h in range(HPC) for p in (0, 1) for w in PASS_WINS[p]]


def _build():
    nc = bacc.Bacc("TRN2", target_bir_lowering=False, debug=False,
                   num_devices=NCORES)
    qt_d = nc.dram_tensor("qt", [HPC, D, S], DT.bfloat16, kind="ExternalInput").ap()
    kt_d = nc.dram_tensor("kt", [HPC, D, S], DT.bfloat16, kind="ExternalInput").ap()
    vp_d = nc.dram_tensor("vp", [HPC, P, NCH, DV], DT.bfloat16, kind="ExternalInput").ap()
    bb_d = nc.dram_tensor("bb", [NCH, P, S], DT.bfloat16, kind="ExternalInput").ap()
    ident_d = nc.dram_tensor("ident", [P, P], DT.bfloat16, kind="ExternalInput").ap()
    cw_d = nc.dram_tensor("cw", [P, CW_TOT], DT.float32, kind="ExternalInput").ap()
    out_d = nc.dram_tensor("out", [HPC, P, NCH, D], DT.float32, kind="ExternalOutput").ap()

    with tile.TileContext(nc) as tc:
        with (
            tc.tile_pool(name="cst", bufs=1) as cst_pool,
            tc.tile_pool(name="qk", bufs=3) as qk_pool,
            tc.tile_pool(name="vw", bufs=2) as v_pool,
            tc.tile_pool(name="ex", bufs=3) as ex_pool,
            tc.tile_pool(name="fx", bufs=3) as fx_pool,
            tc.tile_pool(name="fin", bufs=2) as fin_pool,
            tc.tile_pool(name="pss", bufs=3, space="PSUM") as ps_pool,
            tc.tile_pool(name="pso", bufs=1, space="PSUM") as po_pool,
        ):
            ident_t = cst_pool.tile([P, P], DT.bfloat16, tag="ident")
            bb_t = cst_pool.tile([P, NCH, S], DT.bfloat16, tag="bb")
            ct = {}
            for key, off in CW_OFF.items():
                ct[key] = cst_pool.tile([P, 1024], DT.float32,
                                        tag=f"c{key[0]}_{key[1]}",
                                        name=f"c{key[0]}_{key[1]}")

            heads = {}

            def alloc_head(h):
                qt_t = qk_pool.tile([P, S], DT.bfloat16, tag="qt")
                kt_t = qk_pool.tile([P, S], DT.bfloat16, tag="kt")
                v_t = v_pool.tile([P, NCH, DV], DT.bfloat16, tag="vp")
                dma = nc.sync.dma_start

                def qk_span(x, y):
                    dma(qt_t[0:D, x:y], qt_d[h][:, x:y])
                    dma(qt_t[D:P, x:y], qt_d[h][:, x:y])
                    dma(kt_t[0:D, x:y], kt_d[h][:, x:y])
                    dma(kt_t[D:P, x:y], kt_d[h][:, x:y])

                if h == 0:
                    # boot: land exactly what the first windows need, in
                    # window order -- (0,512) fused needs qt[512:1024] +
                    # its C table; (0,0) needs qt[0:512] + bias chunks
                    # 0/1.  The scalar engine's hardware DGE issues the
                    # early tables in parallel with the sync queue.
                    sdma = nc.scalar.dma_start
                    qk_span(512, 1024)
                    sdma(ct[(0, 512)][:],
                         cw_d[:, CW_OFF[(0, 512)]:CW_OFF[(0, 512)] + 1024])
                    sdma(ident_t[:], ident_d[:])
                    qk_span(0, 512)
                    for c in (0, 1):
                        sdma(bb_t[:, c, :], bb_d[c])
                    dma(v_t[:], vp_d[h])
                    sdma(ct[(2, 512)][:],
                         cw_d[:, CW_OFF[(2, 512)]:CW_OFF[(2, 512)] + 1024])
                    for c in (2, 3):
                        sdma(bb_t[:, c, :], bb_d[c])
                    qk_span(1024, S)
                    for c in (4, 5, 6, 7):
                        dma(bb_t[:, c, :], bb_d[c])
                    for key in DVE_WINS:
                        if key in ((0, 512), (2, 512)):
                            continue
                        dma(ct[key][:], cw_d[:, CW_OFF[key]:CW_OFF[key] + 1024])
                    for c in (8, 9, 10, 11):
                        dma(bb_t[:, c, :], bb_d[c])
                    for c in (12, 13, 14, 15):
                        dma(bb_t[:, c, :], bb_d[c])
                else:
                    qk_span(0, S)
                    dma(v_t[:], vp_d[h])
                heads[h] = (qt_t, kt_t, v_t)

            live_ps = {}

            def emit_qk(h, win):
                (c0, a0, b0, a1, b1) = win
                qt_t, kt_t, _ = heads[h]
                u0, u1 = b0 - a0, b1 - a1
                g0 = 512 - u0
                act = (c0, a0) not in DVE_SET
                ps = ps_pool.tile([P, 1024], DT.float32, tag="st")
                # chunk c0 -> [g0, 512) (bank 0) from PE rows 0-63; chunk
                # c1 -> [512, 512+u1) (bank 1) from rows 64-127; for ACT
                # windows the fp8 DoubleRow bias matmuls then accumulate
                # 184.66*bias^T into each bank (with the causal mask baked
                # into b8 at the diagonal blocks)
                nc.tensor.matmul(
                    ps[:, g0:512],
                    kt_t[0:D, P * c0:P * (c0 + 1)],
                    qt_t[0:D, a0:b0],
                    start=True, stop=not act,
                )
                nc.tensor.matmul(
                    ps[:, 512:512 + u1],
                    kt_t[D:P, P * (c0 + 1):P * (c0 + 2)],
                    qt_t[D:P, a1:b1],
                    start=True, stop=not act,
                )
                if act:
                    nc.tensor.matmul(
                        ps[:, g0:512], ident_t[:], bb_t[:, c0, a0:b0],
                        start=False, stop=True, skip_group_check=True,
                    )
                    nc.tensor.matmul(
                        ps[:, 512:512 + u1],
                        ident_t[:], bb_t[:, c0 + 1, a1:b1],
                        start=False, stop=True, skip_group_check=True,
                    )
                live_ps[(h, c0, a0)] = ps

            accs = {}

            alloc_head(0)
            emit_qk(*(lambda it: (it[0], it[2]))(ITEMS[0]))
            emit_qk(*(lambda it: (it[0], it[2]))(ITEMS[1]))
            for idx, (h, pss, win) in enumerate(ITEMS):
                first_of_pass = idx == 0 or ITEMS[idx - 1][:2] != (h, pss)
                last_of_pass = (idx + 1 == len(ITEMS)
                                or ITEMS[idx + 1][:2] != (h, pss))
                if first_of_pass:
                    if pss == 0 and h + 1 < HPC:
                        alloc_head(h + 1)
                    # per-pass PV accumulators: 8 slots of [128, 65]
                    # packed 7 + 1 in 2 PSUM banks
                    oa = po_pool.tile([P, 7, DV], DT.float32, tag="oa")
                    ob = po_pool.tile([P, 1, DV], DT.float32, tag="ob")
                    accs[(h, pss)] = (oa, ob)
                if idx + 2 < len(ITEMS):
                    nh, _, nwin = ITEMS[idx + 2]
                    emit_qk(nh, nwin)

                (c0, a0, b0, a1, b1) = win
                key = (c0, a0)
                u0, u1 = b0 - a0, b1 - a1
                g0 = 512 - u0
                w = 512 + u1
                ps = live_ps.pop((h, c0, a0))

                if key in DVE_SET:
                    it = fx_pool.tile([P, 1024], DT.int16, tag="fx")
                    nc.vector.tensor_tensor(
                        it[:, g0:w], ps[:, g0:w],
                        ct[key][:, 0:w - g0], mybir.AluOpType.add,
                    )
                    pts = it[:].bitcast(DT.bfloat16)
                else:
                    ex = ex_pool.tile([P, 1024], DT.bfloat16, tag="ex")
                    nc.scalar.activation(
                        ex[:, g0:w], ps[:, g0:w], AF.Exp, scale=ACT_SCALE)
                    pts = ex[:]

                oa, ob = accs[(h, pss)]
                qb0 = 8 * pss

                def oslot(qb):
                    r = qb - qb0
                    return oa[:, r, :] if r < 7 else ob[:, 0, :]

                v_t = heads[h][2]
                starts, stops = PV_FLAGS[pss]
                wi = PASS_WINS[pss].index(win)
                for (c, aa, bb_, toff) in ((c0, a0, b0, g0),
                                           (c0 + 1, a1, b1, 512)):
                    for qb in range(aa // P, bb_ // P):
                        nc.tensor.matmul(
                            oslot(qb),
                            pts[:, toff + qb * P - aa:
                                toff + qb * P - aa + P],
                            v_t[:, c, :],
                            start=(wi, c, qb) in starts,
                            stop=(wi, c, qb) in stops,
                            skip_group_check=True,
                        )

                if last_of_pass:
                    oa, ob = accs.pop((h, pss))
                    rec = fin_pool.tile([P, 8], DT.float32, tag="rec")
                    nc.vector.reciprocal(rec[:, 0:7], oa[:, :, D])
                    nc.vector.reciprocal(rec[:, 7:8], ob[:, :, D])
                    outf = fin_pool.tile([P, 8, D], DT.float32, tag="outf")
                    for (acc, lo, hi) in ((oa, 0, 7), (ob, 7, 8)):
                        a, bb = bass.broadcast_tensor_aps(
                            acc[:, 0:hi - lo, 0:D],
                            rec[:, lo:hi].rearrange("p (n o) -> p n o", o=1),
                        )
                        nc.vector.tensor_tensor(
                            outf[:, lo:hi, :], a, bb, mybir.AluOpType.mult)
                    nc.sync.dma_start(
                        out_d[h][:, 8 * pss:8 * pss + 8, :], outf[:])
                    if pss == 1:
                        heads.pop(h)

    nc.finalize()
    return nc


def kernel(queries, keys, values, queries_mask, values_mask, bias):
    global _built, LAST_EXEC_NS
    q = np.asarray(queries, dtype=np.float32)
    k = np.asarray(keys, dtype=np.float32)
    v = np.asarray(values, dtype=np.float32)
    bias = np.asarray(bias, dtype=np.float32)

    qT = np.ascontiguousarray(
        (q * ALPHA).transpose(0, 1, 3, 2)).astype(ml_dtypes.bfloat16)
    kT = np.ascontiguousarray(
        k.transpose(0, 1, 3, 2)).astype(ml_dtypes.bfloat16)  # [B,H,D,S]
    vp = np.ones((B, H, S, DV), dtype=ml_dtypes.bfloat16)
    vp[..., :D] = v.astype(ml_dtypes.bfloat16)
    # [B,H,P,NCH,DV] so the device DMA is fully contiguous
    vp = np.ascontiguousarray(
        vp.reshape(B, H, NCH, P, DV).transpose(0, 1, 3, 2, 4))

    btf = np.ascontiguousarray(bias[:, 0].transpose(0, 2, 1))  # [B,S,S] (k,q)
    # bias-add moving operand [B, NCH, P, S]: CB_MUL*biasT in bf16, with
    # the upper triangle (k > q) of each diagonal block masked to -40000
    # (exp underflows to exactly 0)
    bb = (CB_MUL * btf).reshape(B, NCH, P, S)
    m = np.arange(P)[:, None] > np.arange(P)[None, :]   # in-chunk k > q
    for c in range(NCH):
        blk = bb[:, c, :, P * c:P * (c + 1)]
        blk[np.broadcast_to(m[None], blk.shape)] = -40000.0
    bb = bb.astype(ml_dtypes.bfloat16)
    ident = np.eye(P, dtype=ml_dtypes.bfloat16)
    # host-precomputed Schraudolph bias bits for the DVE windows
    cw = np.empty((B, P, CW_TOT), dtype=np.float32)
    for (c0, a0), off in CW_OFF.items():
        for half in (0, 1):
            rows = slice(P * (c0 + half), P * (c0 + half + 1))
            cols = slice(off + 512 * half, off + 512 * (half + 1))
            cw[:, :, cols] = CB_MUL * btf[:, rows, a0:a0 + 512] + CB_ADD

    if _built is None:
        _built = _build()
    nc = _built

    in_maps = []
    for c in range(NCORES):
        b, h0 = c // 2, (c % 2) * HPC
        in_maps.append({
            "qt": np.ascontiguousarray(qT[b, h0:h0 + HPC]),
            "kt": np.ascontiguousarray(kT[b, h0:h0 + HPC]),
            "vp": np.ascontiguousarray(vp[b, h0:h0 + HPC]),
            "bb": bb[b],
            "ident": ident,
            "cw": cw[b],
        })

    global LAST_PROFILE_DIR
    if TRACE:
        res, LAST_EXEC_NS, LAST_PROFILE_DIR = _nrt_profile_run(nc, in_maps)
    else:
        res = run_bass_kernel_spmd(nc, in_maps, core_ids=list(range(NCORES)))
        LAST_EXEC_NS = None

    out = np.empty((B, H, S, D), dtype=np.float32)
    for c in range(NCORES):
        b, h0 = c // 2, (c % 2) * HPC
        r = np.asarray(res.results[c]["out"])
        out[b, h0:h0 + HPC] = r.transpose(0, 2, 1, 3).reshape(HPC, S, D)
    return out
